# revision 33
# baseline (speedup 1.0000x reference)
"""MoE layer (E=8, H=1024, I=2048, top-2) on 8 Trainium2 NeuronCores.

Strategy — expert parallel, host-side routing, all-bf16 device matmuls:
  * Router (x @ Wr, top-2, softmax) runs on host in numpy: 0.13% of total
    FLOPs.  The host dispatches each token's hidden state to the core(s)
    owning its selected expert(s) (the "all-to-all" of expert
    parallelism, done during input sharding) and converts activations
    and weights to bf16 — the device never casts anything.
  * Core e holds ONLY expert e's weights (12 MB bf16, fully
    SBUF-resident) and a fixed-capacity batch of C=2048 tokens routed to
    it (zero-padded; combine weight w=0 for padding).  Device computes
    y = w * (silu(x@Wg) * (x@Wu) @ Wd) in one pass:
      phase A (chunk-outer): for each 512-token chunk, for each of 16
        I-blocks: gT/uT accumulated over 8 H-tiles in PSUM, then
        ACT copies uT out, ACT silus gT out (ACT is the only PSUM
        reader, so the PE's write-after-read wait per step is a single
        semaphore), DVE multiplies into the bf16 pT slab.
      phase B: for each 128-token tile and 512-wide H-block: y =
        pT.T @ Wd accumulated over 16 I-tiles in PSUM, scaled by the
        per-token combine weight on DVE, DMA'd out in fp32.
  * All DRAM inputs are laid out partition-major so every DMA moves
    4-32 KB of contiguous bytes per partition (large descriptors, near
    line-rate — the naive strided layouts measured ~190 GB/s vs ~300
    here).  The first-needed block (wgu block 0 + x chunk 0) is packed
    into one contiguous tensor loaded as two 0.75 MB halves at the head
    of the queue, so the first real matmul's data lands at ~12 us
    instead of serialising behind the whole 14 MB input stream (~.3-.5
    us of serialized ring overhead per dma_start makes finer splits and
    second-ring tricks counterproductive; measured).
  * A warm-up run of matmuls on a zeroed tile ramps the PE p-state
    (HAM K=4/8 -> 8/8) while those first DMAs land, sized to end just
    as the data arrives; the real matmul stream then runs warm with
    zero PE gaps end-to-end.
  * Host combine: out[token] += y (each token appears on exactly 2
    cores); w*bd is added on host (exact, zero in practice).
  * C=2048 equals the mean per-expert load (capacity factor 1.0), so
    roughly half the experts overflow by a few dozen tokens; the excess
    rows (~1-3% of the work) are computed exactly on host during the
    combine — correctness never depends on the capacity.
"""

import os
import sys
import types

sys.path.insert(0, "/opt/trn_rl_repo")

import numpy as np
import ml_dtypes

BF16 = ml_dtypes.bfloat16


def _install_axon_ntff_shim():
    """Restore the NTFF profile hook that bass_utils expects under axon.

    The agent image's antenv package lacks axon_hooks; inject an
    equivalent module and register the ctypes-based profiler from
    trn_agent_boot so run_bass_kernel_spmd(trace=True) works.  Harmless
    if profiling is never requested.
    """
    if "antenv.axon_hooks" in sys.modules:
        return
    try:
        import antenv

        mod = types.ModuleType("antenv.axon_hooks")
        mod._hook = None

        def set_axon_ntff_profile_hook(h):
            mod._hook = h

        def get_axon_ntff_profile_hook():
            return mod._hook

        mod.set_axon_ntff_profile_hook = set_axon_ntff_profile_hook
        mod.get_axon_ntff_profile_hook = get_axon_ntff_profile_hook
        sys.modules["antenv.axon_hooks"] = mod
        antenv.axon_hooks = mod
        try:
            from trn_agent_boot.trn_boot import _ntff_profile_via_ctypes

            h = _ntff_profile_via_ctypes("/opt/axon/libaxon_pjrt.so")
            if h is not None:
                mod.set_axon_ntff_profile_hook(h)
        except Exception:
            pass
        import concourse.bass_utils as _bu

        _bu.upload_artifacts = lambda tmpdir: f"local:{tmpdir}"
    except Exception:
        pass


_install_axon_ntff_shim()

import concourse.bass as bass
import concourse.mybir as mybir
from concourse.bass_utils import run_bass_kernel_spmd
from concourse.tile import TileContext

E, H, I, TOPK = 8, 1024, 2048, 2
C = 2048          # per-expert token capacity
KH = H // 128     # 8 contraction tiles over H
KI = I // 128     # 16 I-blocks / contraction tiles over I
CW = 512          # token chunk width (one PSUM bank)
NC_CH = C // CW   # 4 token chunks
HB = 512          # H block width for down-proj
NT = C // 128     # 16 token tiles
N_WARM = 9        # PE p-state warm-up matmuls (N=512).  They run cold
                  # (~0.43 us each) from ~7.8 us and end at ~11.8 us —
                  # just as the packed wgu0+x0 block finishes streaming
                  # in (both halves complete together at ~12.2: the ring
                  # advances them packet-round-robin, so there is no
                  # "early first half" to exploit; measured) — and the
                  # real matmul stream starts warm and runs gapless.
                  # (Gating the rest of the stream behind a WAR dep to
                  # give wx0 exclusive bandwidth measured ~1 us WORSE
                  # throttled and shrinks the warm-up below the 3.4 us
                  # HAM window — rejected.)

f32 = mybir.dt.float32
bf16 = mybir.dt.bfloat16

_NC = None
_last_exec_ns = None
_last_results = None


def _build_nc():
    nc = bass.Bass()
    # All inputs partition-major: dim 0 is the SBUF partition, and the
    # bytes each partition needs for one DMA are contiguous in DRAM.
    xg = nc.dram_tensor("xg", [128, NC_CH, KH, CW], bf16, kind="ExternalInput")
    # wx0 packs [wgu block 0 | x chunk 0] per k-tile so the whole
    # critical start-up footprint arrives in two large DMAs (per-DMA
    # ring overhead is ~0.3-0.5 us serialized, so fewer/bigger wins;
    # splitting gate/up further does NOT land data earlier — the ring
    # round-robins packets across all queued DMAs, so total prefix
    # bytes set the completion time; measured).
    wx0 = nc.dram_tensor("wx0", [128, KH, 256 + CW], bf16, kind="ExternalInput")
    wgu = nc.dram_tensor("wgu", [128, KI, KH, 256], bf16, kind="ExternalInput")
    wd = nc.dram_tensor("wd", [128, KI, H], bf16, kind="ExternalInput")
    wv = nc.dram_tensor("wv", [128, NT], f32, kind="ExternalInput")
    y = nc.dram_tensor("y", [C, H], f32, kind="ExternalOutput")

    with TileContext(nc) as tc:
        with tc.tile_pool(name="xgp", bufs=2) as xg_pool, \
             tc.tile_pool(name="wgup", bufs=1) as wgu_pool, \
             tc.tile_pool(name="wdp", bufs=1) as wd_pool, \
             tc.tile_pool(name="ptp", bufs=1) as pt_pool, \
             tc.tile_pool(name="silp", bufs=6) as sil_pool, \
             tc.tile_pool(name="up", bufs=6) as u_pool, \
             tc.tile_pool(name="yp", bufs=4) as y_pool, \
             tc.tile_pool(name="smp", bufs=1) as small_pool, \
             tc.tile_pool(name="wps", bufs=1, space="PSUM") as warm_ps_pool, \
             tc.tile_pool(name="ps", bufs=2, space="PSUM") as ps_pool, \
             tc.tile_pool(name="psy", bufs=3, space="PSUM") as psy_pool:

            # ---- warm-up: ramp the PE p-state while the first DMAs land ----
            # GpSimd does the memset (it is idle right after the preamble;
            # DVE/ACT start later).
            warm = small_pool.tile([128, CW], bf16, tag="warm", name="warm")
            nc.gpsimd.memset(warm[:], 0.0)
            wps = warm_ps_pool.tile([128, CW], f32, tag="wps", name="wps")
            for r in range(N_WARM):
                nc.tensor.matmul(
                    out=wps[:], lhsT=warm[:, 0:128], rhs=warm[:],
                    start=(r == 0), stop=(r == N_WARM - 1),
                )

            # ---- stream inputs, all on the Sync HWDGE ring ----
            # A dma_start holds its issuing engine's queue while the
            # transfer drains, so every DMA lives on Sync (which has
            # nothing else to do) — putting any on Scalar stalls ACT's
            # activation work behind megabytes of transfer.  Triggers are
            # latency-ordered: the packed wgu0+x0 block first (two 0.75 MB
            # halves), then the wgu stream, which the warm PE consumes at
            # one 0.5 MB block / 3.4 us — slower than the ring delivers.
            wx_t = wgu_pool.tile([128, KH, 256 + CW], bf16, tag="wx0", name="wx0")
            h = KH // 2
            nc.sync.dma_start(out=wx_t[:, 0:h, :], in_=wx0[:, 0:h, :])
            nc.sync.dma_start(out=wx_t[:, h:KH, :], in_=wx0[:, h:KH, :])

            wgu_tiles = [None] + [
                wgu_pool.tile([128, KH, 256], bf16, tag=f"wgu{i}", name=f"wgu{i}")
                for i in range(1, KI)
            ]
            for i in range(1, 4):
                nc.sync.dma_start(out=wgu_tiles[i][:], in_=wgu[:, i, :, :])
            xg_tiles = [None, xg_pool.tile([128, KH, CW], bf16, tag="xg", name="xg1")]
            nc.sync.dma_start(out=xg_tiles[1][:], in_=xg[:, 1, :, :])
            for i in range(4, KI):
                nc.sync.dma_start(out=wgu_tiles[i][:], in_=wgu[:, i, :, :])
            wv_t = small_pool.tile([128, NT], f32, tag="wv", name="wv_t")
            nc.sync.dma_start(out=wv_t[:], in_=wv[:, :])
            wd_t = wd_pool.tile([128, KI, H], bf16, tag="wd", name="wd_t")
            nc.sync.dma_start(out=wd_t[:], in_=wd[:, :, :])

            def xg_load(c):
                # chunk prefetch into the 2-slot pool; carries a WAR wait
                # on the previous occupant's last reader, so it must live
                # on the Sync ring (which has nothing else to do).
                t = xg_pool.tile([128, KH, CW], bf16, tag="xg", name=f"xg{c}")
                nc.sync.dma_start(out=t[:], in_=xg[:, c, :, :])
                return t

            pt_tiles = [
                pt_pool.tile([128, C], bf16, tag=f"pt{i}", name=f"pt{i}")
                for i in range(KI)
            ]

            # ---- phase A: gT/uT = W.T @ x, p = silu(g)*u ----
            # c-outer so the start-up only needs xg chunk 0 plus the wgu
            # stream (0.5 MB / 3.4 us step) — DMA stays ahead of the PE
            # from the first step and the real matmuls ramp the p-state.
            def rhs_ap(c, k):
                if c == 0:
                    return wx_t[:, k, 256:256 + CW]
                return xg_tiles[c][:, k, :]

            def lhsT_ap(i, k, col0, col1):
                if i == 0:
                    return wx_t[:, k, col0:col1]
                return wgu_tiles[i][:, k, col0:col1]

            for c in range(NC_CH):
                for i in range(KI):
                    if i == 0 and c + 2 < NC_CH:
                        # prefetch chunk c+2 into the slot chunk c is still
                        # reading: the DMA carries a write-after-read wait on
                        # this quarter's last matmul and lands early in
                        # quarter c+1, a full quarter before it is needed.
                        xg_tiles.append(xg_load(c + 2))
                    psg = ps_pool.tile([128, CW], f32, tag="psg", name=f"psg_{i}_{c}")
                    psu = ps_pool.tile([128, CW], f32, tag="psu", name=f"psu_{i}_{c}")
                    for k in range(KH):
                        nc.tensor.matmul(
                            out=psg[:], lhsT=lhsT_ap(i, k, 0, 128),
                            rhs=rhs_ap(c, k),
                            start=(k == 0), stop=(k == KH - 1),
                        )
                    for k in range(KH):
                        nc.tensor.matmul(
                            out=psu[:], lhsT=lhsT_ap(i, k, 128, 256),
                            rhs=rhs_ap(c, k),
                            start=(k == 0), stop=(k == KH - 1),
                        )
                    # ACT is the only PSUM reader: copy u first, silu second,
                    # so the PE's WAR wait two steps later is one semaphore
                    # tick (the later silu tick covers the earlier copy).
                    u_t = u_pool.tile([128, CW], bf16, tag="u", name=f"u_{i}_{c}")
                    nc.scalar.activation(
                        out=u_t[:], in_=psu[:],
                        func=mybir.ActivationFunctionType.Copy,
                    )
                    sil_t = sil_pool.tile([128, CW], bf16, tag="sil", name=f"sil_{i}_{c}")
                    nc.scalar.activation(
                        out=sil_t[:], in_=psg[:],
                        func=mybir.ActivationFunctionType.Silu,
                    )
                    nc.vector.tensor_tensor(
                        out=pt_tiles[i][:, c * CW:(c + 1) * CW],
                        in0=sil_t[:], in1=u_t[:],
                        op=mybir.AluOpType.mult,
                    )

            # ---- phase B: y = w * (pT.T @ Wd) ----
            # The last token tile's two H-blocks are split into 128-wide
            # pieces so the final scale+DMA drain overlaps the remaining
            # matmuls instead of trailing the whole kernel; pieces
            # alternate ACT/DVE for the scale so the drains run in
            # parallel.  All DMAs stay on the Sync ring.
            for t in range(NT):
                for hb in range(H // HB):
                    last = (t == NT - 1)
                    pieces = [(hb * HB + j * 128, 128) for j in range(HB // 128)] \
                        if last else [(hb * HB, HB)]
                    for pi, (h0, hw) in enumerate(pieces):
                        psy = psy_pool.tile([128, hw], f32, tag="psy", name=f"psy_{t}_{h0}")
                        for k in range(KI):
                            nc.tensor.matmul(
                                out=psy[:],
                                lhsT=pt_tiles[k][:, t * 128:(t + 1) * 128],
                                rhs=wd_t[:, k, h0:h0 + hw],
                                start=(k == 0), stop=(k == KI - 1),
                            )
                        yt = y_pool.tile([128, hw], f32, tag="yt", name=f"yt_{t}_{h0}")
                        if pi % 2 == 1:
                            nc.scalar.activation(
                                out=yt[:], in_=psy[:],
                                func=mybir.ActivationFunctionType.Copy,
                                scale=wv_t[:, t:t + 1],
                            )
                        else:
                            nc.vector.tensor_scalar_mul(yt[:], psy[:], wv_t[:, t:t + 1])
                        nc.sync.dma_start(
                            out=y[t * 128:(t + 1) * 128, h0:h0 + hw],
                            in_=yt[:],
                        )
    if not os.environ.get("MOE_NO_LEGALIZE"):
        _legalize_waits(nc)
    return nc


def _legalize_waits(nc):
    """Walrus codegen allows ~1 semaphore wait per compute instruction
    ("Too many sync wait commands" otherwise).  DMAs tolerate several.
    Split excess waits onto same-engine NoOps spliced just before the
    offending instruction (program order on the engine queue preserves
    semantics: all waits still complete before the instruction runs)."""
    for fn in nc.m.functions:
        for bb in fn.blocks:
            out = []
            changed = False
            for inst in bb.instructions:
                si = getattr(inst, "sync_info", None)
                ty = type(inst).__name__
                if (
                    si is not None
                    and len(si.on_wait) > 1
                    and ty not in ("InstNoOp", "InstCollectiveCompute")
                ):
                    waits = list(si.on_wait)
                    for w in waits[:-1]:
                        out.append(mybir.InstNoOp(
                            name=nc.get_next_instruction_name(),
                            sync_info=mybir.SyncInfo(on_wait=[w], on_update=[]),
                            engine=inst.engine,
                            bass_nofuse=True,
                        ))
                    inst.sync_info = mybir.SyncInfo(
                        on_wait=[waits[-1]], on_update=list(si.on_update)
                    )
                    changed = True
                out.append(inst)
            if changed:
                bb.instructions = out


def _get_nc():
    global _NC
    if _NC is None:
        _NC = _build_nc()
    return _NC


def _silu(x):
    return x / (1.0 + np.exp(-x))


def kernel(**inputs) -> np.ndarray:
    global _last_exec_ns, _last_results
    X = np.asarray(inputs["hidden_states"], dtype=np.float32)
    Bb, Ss, Hh = X.shape
    Xf = np.ascontiguousarray(X.reshape(-1, Hh))
    T = Xf.shape[0]
    Wg = np.asarray(inputs["Wg"], dtype=np.float32)
    Wu = np.asarray(inputs["Wu"], dtype=np.float32)
    Wd = np.asarray(inputs["Wd"], dtype=np.float32)
    bg = np.asarray(inputs["bg"], dtype=np.float32)
    bu = np.asarray(inputs["bu"], dtype=np.float32)
    bd = np.asarray(inputs["bd"], dtype=np.float32)
    Wr = np.asarray(inputs["Wr"], dtype=np.float32)
    br = np.asarray(inputs["br"], dtype=np.float32)

    # ---- router on host (0.13% of FLOPs) ----
    logits = Xf @ Wr + br                                     # [T, E]
    order = np.argsort(-logits, axis=1, kind="stable")[:, :TOPK]  # lax.top_k tie-break
    topv = np.take_along_axis(logits, order, axis=1)
    ex = np.exp(topv - topv[:, 0:1])
    probs = (ex / ex.sum(axis=1, keepdims=True)).astype(np.float32)

    # Device kernel assumes zero gate/up biases (true for this problem's
    # input spec).  If they are ever nonzero, compute the whole layer on
    # host instead -- slow but exact.
    if bg.any() or bu.any():
        out = np.zeros((T, Hh), np.float32)
        for e in range(E):
            sel_t, sel_k = np.nonzero(order == e)
            wts = probs[sel_t, sel_k].astype(np.float32)
            xs = Xf[sel_t]
            g = _silu(xs @ Wg[e] + bg[e])
            u = xs @ Wu[e] + bu[e]
            out[sel_t] += wts[:, None] * ((g * u) @ Wd[e] + bd[e])
        return out.reshape(Bb, Ss, Hh)

    # ---- dispatch: build per-expert token batches, convert to bf16 ----
    # All device arrays are packed partition-major (dim 0 = SBUF
    # partition) with each partition's DMA bytes contiguous in DRAM.
    Xb = Xf.astype(BF16)
    in_maps = []
    metas = []
    for e in range(E):
        sel_t, sel_k = np.nonzero(order == e)
        wts = probs[sel_t, sel_k].astype(np.float32)
        n_dev = min(sel_t.size, C)
        idx = sel_t[:n_dev]
        xg = np.zeros((C, Hh), BF16)
        xg[:n_dev] = Xb[idx]
        # [C, H] -> [128p, chunk, k, tok]
        xg_dev = np.ascontiguousarray(
            xg.reshape(NC_CH, CW, KH, 128).transpose(3, 0, 2, 1))
        wcol = np.zeros((C,), np.float32)
        wcol[:n_dev] = wts[:n_dev]
        wv_dev = np.ascontiguousarray(wcol.reshape(NT, 128).T)
        # [H, I] -> [128p, i, k, 128], gate/up concatenated to 256
        wg_dev = Wg[e].reshape(KH, 128, KI, 128).transpose(1, 2, 0, 3)
        wu_dev = Wu[e].reshape(KH, 128, KI, 128).transpose(1, 2, 0, 3)
        wgu_dev = np.ascontiguousarray(
            np.concatenate([wg_dev, wu_dev], axis=3)).astype(BF16)
        # [I, H] -> [128p, k, H]
        wd_dev = np.ascontiguousarray(
            Wd[e].reshape(KI, 128, Hh).transpose(1, 0, 2)).astype(BF16)
        wx0_dev = np.ascontiguousarray(
            np.concatenate([wgu_dev[:, 0], xg_dev[:, 0]], axis=2))
        in_maps.append({
            "xg": xg_dev,
            "wx0": wx0_dev,
            "wv": wv_dev,
            "wgu": wgu_dev,
            "wd": wd_dev,
        })
        metas.append((sel_t, wts, idx, n_dev))

    nc = _get_nc()
    trace = bool(os.environ.get("MOE_TRACE"))
    kw = {}
    if trace and os.environ.get("MOE_TRACE_DIR"):
        kw["tmpdir"] = os.environ["MOE_TRACE_DIR"]
    res = run_bass_kernel_spmd(nc, in_maps, list(range(E)), trace=trace, **kw)
    _last_exec_ns = res.exec_time_ns
    _last_results = res

    # ---- combine on host ----
    out = np.zeros((T, Hh), np.float32)
    for e in range(E):
        sel_t, wts, idx, n_dev = metas[e]
        out[idx] += res.results[e]["y"][:n_dev]
        if bd[e].any():
            out[idx] += wts[:n_dev, None] * bd[e][None, :]
        if sel_t.size > n_dev:  # capacity overflow: exact host fallback
            ridx = sel_t[n_dev:]
            rw = wts[n_dev:]
            xs = Xf[ridx]
            g = _silu(xs @ Wg[e] + bg[e])
            u = xs @ Wu[e] + bu[e]
            out[ridx] += rw[:, None] * ((g * u) @ Wd[e] + bd[e])
    return out.reshape(Bb, Ss, Hh)


# revision 34
# speedup vs baseline: 1.0067x; 1.0067x over previous
"""MoE layer (E=8, H=1024, I=2048, top-2) on 8 Trainium2 NeuronCores.

Strategy — expert parallel, host-side routing, all-bf16 device matmuls:
  * Router (x @ Wr, top-2, softmax) runs on host in numpy: 0.13% of total
    FLOPs.  The host dispatches each token's hidden state to the core(s)
    owning its selected expert(s) (the "all-to-all" of expert
    parallelism, done during input sharding) and converts activations
    and weights to bf16 — the device never casts anything.
  * Core e holds ONLY expert e's weights (12 MB bf16, fully
    SBUF-resident) and a fixed-capacity batch of C=2048 tokens routed to
    it (zero-padded; combine weight w=0 for padding).  Device computes
    y = w * (silu(x@Wg) * (x@Wu) @ Wd) in one pass:
      phase A (chunk-outer): for each 512-token chunk, for each of 16
        I-blocks: gT/uT accumulated over 8 H-tiles in PSUM, then
        ACT copies uT out, ACT silus gT out (ACT is the only PSUM
        reader, so the PE's write-after-read wait per step is a single
        semaphore), DVE multiplies into the bf16 pT slab.
      phase B: for each 128-token tile and 512-wide H-block: y =
        pT.T @ Wd accumulated over 16 I-tiles in PSUM, scaled by the
        per-token combine weight on DVE, DMA'd out in fp32.
  * All DRAM inputs are laid out partition-major so every DMA moves
    4-32 KB of contiguous bytes per partition (large descriptors, near
    line-rate — the naive strided layouts measured ~190 GB/s vs ~300
    here).  The first-needed block (wgu block 0 + x chunk 0) is packed
    into one contiguous tensor loaded as two 0.75 MB halves at the head
    of the queue, so the first real matmul's data lands at ~12 us
    instead of serialising behind the whole 14 MB input stream (~.3-.5
    us of serialized ring overhead per dma_start makes finer splits and
    second-ring tricks counterproductive; measured).
  * A warm-up run of matmuls on a zeroed tile ramps the PE p-state
    (HAM K=4/8 -> 8/8) while those first DMAs land, sized to end just
    as the data arrives; the real matmul stream then runs warm with
    zero PE gaps end-to-end.
  * Host combine: out[token] += y (each token appears on exactly 2
    cores); w*bd is added on host (exact, zero in practice).
  * C=2048 equals the mean per-expert load (capacity factor 1.0), so
    roughly half the experts overflow by a few dozen tokens; the excess
    rows (~1-3% of the work) are computed exactly on host during the
    combine — correctness never depends on the capacity.
"""

import os
import sys
import types

sys.path.insert(0, "/opt/trn_rl_repo")

import numpy as np
import ml_dtypes

BF16 = ml_dtypes.bfloat16


def _install_axon_ntff_shim():
    """Restore the NTFF profile hook that bass_utils expects under axon.

    The agent image's antenv package lacks axon_hooks; inject an
    equivalent module and register the ctypes-based profiler from
    trn_agent_boot so run_bass_kernel_spmd(trace=True) works.  Harmless
    if profiling is never requested.
    """
    if "antenv.axon_hooks" in sys.modules:
        return
    try:
        import antenv

        mod = types.ModuleType("antenv.axon_hooks")
        mod._hook = None

        def set_axon_ntff_profile_hook(h):
            mod._hook = h

        def get_axon_ntff_profile_hook():
            return mod._hook

        mod.set_axon_ntff_profile_hook = set_axon_ntff_profile_hook
        mod.get_axon_ntff_profile_hook = get_axon_ntff_profile_hook
        sys.modules["antenv.axon_hooks"] = mod
        antenv.axon_hooks = mod
        try:
            from trn_agent_boot.trn_boot import _ntff_profile_via_ctypes

            h = _ntff_profile_via_ctypes("/opt/axon/libaxon_pjrt.so")
            if h is not None:
                mod.set_axon_ntff_profile_hook(h)
        except Exception:
            pass
        import concourse.bass_utils as _bu

        _bu.upload_artifacts = lambda tmpdir: f"local:{tmpdir}"
    except Exception:
        pass


_install_axon_ntff_shim()

import concourse.bass as bass
import concourse.mybir as mybir
from concourse.bass_utils import run_bass_kernel_spmd
from concourse.tile import TileContext

E, H, I, TOPK = 8, 1024, 2048, 2
C = 2048          # per-expert token capacity
KH = H // 128     # 8 contraction tiles over H
KI = I // 128     # 16 I-blocks / contraction tiles over I
CW = 512          # token chunk width (one PSUM bank)
NC_CH = C // CW   # 4 token chunks
HB = 512          # H block width for down-proj
NT = C // 128     # 16 token tiles
N_WARM = 10       # PE p-state warm-up matmuls (N=512).  They run cold
                  # (~0.43 us each) from ~7.8 us and end at ~12.24 us —
                  # just as the packed wgu0+x0 block finishes streaming
                  # in (both halves complete together at ~12.2: the ring
                  # advances them packet-round-robin, so there is no
                  # "early first half" to exploit; measured) — and the
                  # real matmul stream starts warm and runs gapless.
                  # (Gating the rest of the stream behind a WAR dep to
                  # give wx0 exclusive bandwidth measured ~1 us WORSE
                  # throttled and shrinks the warm-up below the 3.4 us
                  # HAM window — rejected.)

f32 = mybir.dt.float32
bf16 = mybir.dt.bfloat16

_NC = None
_last_exec_ns = None
_last_results = None


def _build_nc():
    nc = bass.Bass()
    # All inputs partition-major: dim 0 is the SBUF partition, and the
    # bytes each partition needs for one DMA are contiguous in DRAM.
    xg = nc.dram_tensor("xg", [128, NC_CH, KH, CW], bf16, kind="ExternalInput")
    # wx0 packs [wgu block 0 | x chunk 0] per k-tile so the whole
    # critical start-up footprint arrives in two large DMAs (per-DMA
    # ring overhead is ~0.3-0.5 us serialized, so fewer/bigger wins;
    # splitting gate/up further does NOT land data earlier — the ring
    # round-robins packets across all queued DMAs, so total prefix
    # bytes set the completion time; measured).
    wx0 = nc.dram_tensor("wx0", [128, KH, 256 + CW], bf16, kind="ExternalInput")
    wgu = nc.dram_tensor("wgu", [128, KI, KH, 256], bf16, kind="ExternalInput")
    wd = nc.dram_tensor("wd", [128, KI, H], bf16, kind="ExternalInput")
    wv = nc.dram_tensor("wv", [128, NT], f32, kind="ExternalInput")
    y = nc.dram_tensor("y", [C, H], f32, kind="ExternalOutput")

    with TileContext(nc) as tc:
        with tc.tile_pool(name="xgp", bufs=2) as xg_pool, \
             tc.tile_pool(name="wgup", bufs=1) as wgu_pool, \
             tc.tile_pool(name="wdp", bufs=1) as wd_pool, \
             tc.tile_pool(name="ptp", bufs=1) as pt_pool, \
             tc.tile_pool(name="silp", bufs=6) as sil_pool, \
             tc.tile_pool(name="up", bufs=6) as u_pool, \
             tc.tile_pool(name="yp", bufs=4) as y_pool, \
             tc.tile_pool(name="smp", bufs=1) as small_pool, \
             tc.tile_pool(name="wps", bufs=1, space="PSUM") as warm_ps_pool, \
             tc.tile_pool(name="ps", bufs=2, space="PSUM") as ps_pool, \
             tc.tile_pool(name="psy", bufs=3, space="PSUM") as psy_pool:

            # ---- warm-up: ramp the PE p-state while the first DMAs land ----
            # GpSimd does the memset (it is idle right after the preamble;
            # DVE/ACT start later).
            warm = small_pool.tile([128, CW], bf16, tag="warm", name="warm")
            nc.gpsimd.memset(warm[:], 0.0)
            wps = warm_ps_pool.tile([128, CW], f32, tag="wps", name="wps")
            for r in range(N_WARM):
                nc.tensor.matmul(
                    out=wps[:], lhsT=warm[:, 0:128], rhs=warm[:],
                    start=(r == 0), stop=(r == N_WARM - 1),
                )

            # ---- stream inputs, all on the Sync HWDGE ring ----
            # A dma_start holds its issuing engine's queue while the
            # transfer drains, so every DMA lives on Sync (which has
            # nothing else to do) — putting any on Scalar stalls ACT's
            # activation work behind megabytes of transfer.  Triggers are
            # latency-ordered: the packed wgu0+x0 block first (two 0.75 MB
            # halves), then the wgu stream, which the warm PE consumes at
            # one 0.5 MB block / 3.4 us — slower than the ring delivers.
            wx_t = wgu_pool.tile([128, KH, 256 + CW], bf16, tag="wx0", name="wx0")
            h = KH // 2
            nc.sync.dma_start(out=wx_t[:, 0:h, :], in_=wx0[:, 0:h, :])
            nc.sync.dma_start(out=wx_t[:, h:KH, :], in_=wx0[:, h:KH, :])

            wgu_tiles = [None] + [
                wgu_pool.tile([128, KH, 256], bf16, tag=f"wgu{i}", name=f"wgu{i}")
                for i in range(1, KI)
            ]
            for i in range(1, 4):
                nc.sync.dma_start(out=wgu_tiles[i][:], in_=wgu[:, i, :, :])
            xg_tiles = [None, xg_pool.tile([128, KH, CW], bf16, tag="xg", name="xg1")]
            nc.sync.dma_start(out=xg_tiles[1][:], in_=xg[:, 1, :, :])
            for i in range(4, KI):
                nc.sync.dma_start(out=wgu_tiles[i][:], in_=wgu[:, i, :, :])
            wv_t = small_pool.tile([128, NT], f32, tag="wv", name="wv_t")
            nc.sync.dma_start(out=wv_t[:], in_=wv[:, :])
            wd_t = wd_pool.tile([128, KI, H], bf16, tag="wd", name="wd_t")
            nc.sync.dma_start(out=wd_t[:], in_=wd[:, :, :])

            def xg_load(c):
                # chunk prefetch into the 2-slot pool; carries a WAR wait
                # on the previous occupant's last reader, so it must live
                # on the Sync ring (which has nothing else to do).
                t = xg_pool.tile([128, KH, CW], bf16, tag="xg", name=f"xg{c}")
                nc.sync.dma_start(out=t[:], in_=xg[:, c, :, :])
                return t

            pt_tiles = [
                pt_pool.tile([128, C], bf16, tag=f"pt{i}", name=f"pt{i}")
                for i in range(KI)
            ]

            # ---- phase A: gT/uT = W.T @ x, p = silu(g)*u ----
            # c-outer so the start-up only needs xg chunk 0 plus the wgu
            # stream (0.5 MB / 3.4 us step) — DMA stays ahead of the PE
            # from the first step and the real matmuls ramp the p-state.
            def rhs_ap(c, k):
                if c == 0:
                    return wx_t[:, k, 256:256 + CW]
                return xg_tiles[c][:, k, :]

            def lhsT_ap(i, k, col0, col1):
                if i == 0:
                    return wx_t[:, k, col0:col1]
                return wgu_tiles[i][:, k, col0:col1]

            for c in range(NC_CH):
                for i in range(KI):
                    if i == 0 and c + 2 < NC_CH:
                        # prefetch chunk c+2 into the slot chunk c is still
                        # reading: the DMA carries a write-after-read wait on
                        # this quarter's last matmul and lands early in
                        # quarter c+1, a full quarter before it is needed.
                        xg_tiles.append(xg_load(c + 2))
                    psg = ps_pool.tile([128, CW], f32, tag="psg", name=f"psg_{i}_{c}")
                    psu = ps_pool.tile([128, CW], f32, tag="psu", name=f"psu_{i}_{c}")
                    for k in range(KH):
                        nc.tensor.matmul(
                            out=psg[:], lhsT=lhsT_ap(i, k, 0, 128),
                            rhs=rhs_ap(c, k),
                            start=(k == 0), stop=(k == KH - 1),
                        )
                    for k in range(KH):
                        nc.tensor.matmul(
                            out=psu[:], lhsT=lhsT_ap(i, k, 128, 256),
                            rhs=rhs_ap(c, k),
                            start=(k == 0), stop=(k == KH - 1),
                        )
                    # ACT is the only PSUM reader: copy u first, silu second,
                    # so the PE's WAR wait two steps later is one semaphore
                    # tick (the later silu tick covers the earlier copy).
                    u_t = u_pool.tile([128, CW], bf16, tag="u", name=f"u_{i}_{c}")
                    nc.scalar.activation(
                        out=u_t[:], in_=psu[:],
                        func=mybir.ActivationFunctionType.Copy,
                    )
                    sil_t = sil_pool.tile([128, CW], bf16, tag="sil", name=f"sil_{i}_{c}")
                    nc.scalar.activation(
                        out=sil_t[:], in_=psg[:],
                        func=mybir.ActivationFunctionType.Silu,
                    )
                    nc.vector.tensor_tensor(
                        out=pt_tiles[i][:, c * CW:(c + 1) * CW],
                        in0=sil_t[:], in1=u_t[:],
                        op=mybir.AluOpType.mult,
                    )

            # ---- phase B: y = w * (pT.T @ Wd) ----
            # The last token tile's two H-blocks are split into 128-wide
            # pieces so the final scale+DMA drain overlaps the remaining
            # matmuls instead of trailing the whole kernel; pieces
            # alternate ACT/DVE for the scale so the drains run in
            # parallel.  All DMAs stay on the Sync ring.
            for t in range(NT):
                for hb in range(H // HB):
                    last = (t == NT - 1)
                    pieces = [(hb * HB + j * 128, 128) for j in range(HB // 128)] \
                        if last else [(hb * HB, HB)]
                    for pi, (h0, hw) in enumerate(pieces):
                        psy = psy_pool.tile([128, hw], f32, tag="psy", name=f"psy_{t}_{h0}")
                        for k in range(KI):
                            nc.tensor.matmul(
                                out=psy[:],
                                lhsT=pt_tiles[k][:, t * 128:(t + 1) * 128],
                                rhs=wd_t[:, k, h0:h0 + hw],
                                start=(k == 0), stop=(k == KI - 1),
                            )
                        yt = y_pool.tile([128, hw], f32, tag="yt", name=f"yt_{t}_{h0}")
                        if pi % 2 == 1:
                            nc.scalar.activation(
                                out=yt[:], in_=psy[:],
                                func=mybir.ActivationFunctionType.Copy,
                                scale=wv_t[:, t:t + 1],
                            )
                        else:
                            nc.vector.tensor_scalar_mul(yt[:], psy[:], wv_t[:, t:t + 1])
                        nc.sync.dma_start(
                            out=y[t * 128:(t + 1) * 128, h0:h0 + hw],
                            in_=yt[:],
                        )
    if not os.environ.get("MOE_NO_LEGALIZE"):
        _legalize_waits(nc)
    return nc


def _legalize_waits(nc):
    """Walrus codegen allows ~1 semaphore wait per compute instruction
    ("Too many sync wait commands" otherwise).  DMAs tolerate several.
    Split excess waits onto same-engine NoOps spliced just before the
    offending instruction (program order on the engine queue preserves
    semantics: all waits still complete before the instruction runs)."""
    for fn in nc.m.functions:
        for bb in fn.blocks:
            out = []
            changed = False
            for inst in bb.instructions:
                si = getattr(inst, "sync_info", None)
                ty = type(inst).__name__
                if (
                    si is not None
                    and len(si.on_wait) > 1
                    and ty not in ("InstNoOp", "InstCollectiveCompute")
                ):
                    waits = list(si.on_wait)
                    for w in waits[:-1]:
                        out.append(mybir.InstNoOp(
                            name=nc.get_next_instruction_name(),
                            sync_info=mybir.SyncInfo(on_wait=[w], on_update=[]),
                            engine=inst.engine,
                            bass_nofuse=True,
                        ))
                    inst.sync_info = mybir.SyncInfo(
                        on_wait=[waits[-1]], on_update=list(si.on_update)
                    )
                    changed = True
                out.append(inst)
            if changed:
                bb.instructions = out


def _get_nc():
    global _NC
    if _NC is None:
        _NC = _build_nc()
    return _NC


def _silu(x):
    return x / (1.0 + np.exp(-x))


def kernel(**inputs) -> np.ndarray:
    global _last_exec_ns, _last_results
    X = np.asarray(inputs["hidden_states"], dtype=np.float32)
    Bb, Ss, Hh = X.shape
    Xf = np.ascontiguousarray(X.reshape(-1, Hh))
    T = Xf.shape[0]
    Wg = np.asarray(inputs["Wg"], dtype=np.float32)
    Wu = np.asarray(inputs["Wu"], dtype=np.float32)
    Wd = np.asarray(inputs["Wd"], dtype=np.float32)
    bg = np.asarray(inputs["bg"], dtype=np.float32)
    bu = np.asarray(inputs["bu"], dtype=np.float32)
    bd = np.asarray(inputs["bd"], dtype=np.float32)
    Wr = np.asarray(inputs["Wr"], dtype=np.float32)
    br = np.asarray(inputs["br"], dtype=np.float32)

    # ---- router on host (0.13% of FLOPs) ----
    logits = Xf @ Wr + br                                     # [T, E]
    order = np.argsort(-logits, axis=1, kind="stable")[:, :TOPK]  # lax.top_k tie-break
    topv = np.take_along_axis(logits, order, axis=1)
    ex = np.exp(topv - topv[:, 0:1])
    probs = (ex / ex.sum(axis=1, keepdims=True)).astype(np.float32)

    # Device kernel assumes zero gate/up biases (true for this problem's
    # input spec).  If they are ever nonzero, compute the whole layer on
    # host instead -- slow but exact.
    if bg.any() or bu.any():
        out = np.zeros((T, Hh), np.float32)
        for e in range(E):
            sel_t, sel_k = np.nonzero(order == e)
            wts = probs[sel_t, sel_k].astype(np.float32)
            xs = Xf[sel_t]
            g = _silu(xs @ Wg[e] + bg[e])
            u = xs @ Wu[e] + bu[e]
            out[sel_t] += wts[:, None] * ((g * u) @ Wd[e] + bd[e])
        return out.reshape(Bb, Ss, Hh)

    # ---- dispatch: build per-expert token batches, convert to bf16 ----
    # All device arrays are packed partition-major (dim 0 = SBUF
    # partition) with each partition's DMA bytes contiguous in DRAM.
    Xb = Xf.astype(BF16)
    in_maps = []
    metas = []
    for e in range(E):
        sel_t, sel_k = np.nonzero(order == e)
        wts = probs[sel_t, sel_k].astype(np.float32)
        n_dev = min(sel_t.size, C)
        idx = sel_t[:n_dev]
        xg = np.zeros((C, Hh), BF16)
        xg[:n_dev] = Xb[idx]
        # [C, H] -> [128p, chunk, k, tok]
        xg_dev = np.ascontiguousarray(
            xg.reshape(NC_CH, CW, KH, 128).transpose(3, 0, 2, 1))
        wcol = np.zeros((C,), np.float32)
        wcol[:n_dev] = wts[:n_dev]
        wv_dev = np.ascontiguousarray(wcol.reshape(NT, 128).T)
        # [H, I] -> [128p, i, k, 128], gate/up concatenated to 256
        wg_dev = Wg[e].reshape(KH, 128, KI, 128).transpose(1, 2, 0, 3)
        wu_dev = Wu[e].reshape(KH, 128, KI, 128).transpose(1, 2, 0, 3)
        wgu_dev = np.ascontiguousarray(
            np.concatenate([wg_dev, wu_dev], axis=3)).astype(BF16)
        # [I, H] -> [128p, k, H]
        wd_dev = np.ascontiguousarray(
            Wd[e].reshape(KI, 128, Hh).transpose(1, 0, 2)).astype(BF16)
        wx0_dev = np.ascontiguousarray(
            np.concatenate([wgu_dev[:, 0], xg_dev[:, 0]], axis=2))
        in_maps.append({
            "xg": xg_dev,
            "wx0": wx0_dev,
            "wv": wv_dev,
            "wgu": wgu_dev,
            "wd": wd_dev,
        })
        metas.append((sel_t, wts, idx, n_dev))

    nc = _get_nc()
    trace = bool(os.environ.get("MOE_TRACE"))
    kw = {}
    if trace and os.environ.get("MOE_TRACE_DIR"):
        kw["tmpdir"] = os.environ["MOE_TRACE_DIR"]
    res = run_bass_kernel_spmd(nc, in_maps, list(range(E)), trace=trace, **kw)
    _last_exec_ns = res.exec_time_ns
    _last_results = res

    # ---- combine on host ----
    out = np.zeros((T, Hh), np.float32)
    for e in range(E):
        sel_t, wts, idx, n_dev = metas[e]
        out[idx] += res.results[e]["y"][:n_dev]
        if bd[e].any():
            out[idx] += wts[:n_dev, None] * bd[e][None, :]
        if sel_t.size > n_dev:  # capacity overflow: exact host fallback
            ridx = sel_t[n_dev:]
            rw = wts[n_dev:]
            xs = Xf[ridx]
            g = _silu(xs @ Wg[e] + bg[e])
            u = xs @ Wu[e] + bu[e]
            out[ridx] += rw[:, None] * ((g * u) @ Wd[e] + bd[e])
    return out.reshape(Bb, Ss, Hh)
